# revision 1
# baseline (speedup 1.0000x reference)
"""Trainium2 Bass kernel for nn_ByteBitwiseFFN.

Reference semantics (per token, D=128 features):
  a = argmax(x[4:20]) + 16*argmax(x[20:36])
  b = argmax(x[36:52]) + 16*argmax(x[52:68])
  res = AND/OR/XOR LUT[a,b] picked by flags x[1]>0.5 / x[2]>0.5 / x[3]>0.5
        (priority AND, OR, XOR; XOR value also used when no flag set)
  active = (x[0]>=0.5) & any-flag; w = active ? 2 : 0
  out = x; out[68 + (res&15)] += w; out[84 + (res>>4)] += w

Key identity: bitwise ops factor over nibbles, so the 256x256 LUTs are
never needed:  res&15 = op(a_lo, b_lo), res>>4 = op(a_hi, b_hi), and for
4-bit operands op(u, v) = alpha*(u+v) + beta*(u AND v) with
(alpha, beta) = (0,1) AND / (1,-1) OR / (1,-2) XOR, where
u AND v = sum_k 2^k u_k v_k over bits.  Everything is elementwise math.

Sharding: pure data parallel over tokens; each of the 8 cores gets
131072/8 = 16384 tokens as its own ExternalInput.
"""

import sys

if "/opt/trn_rl_repo" not in sys.path:
    sys.path.insert(0, "/opt/trn_rl_repo")

import numpy as np

B, S, D = 16, 8192, 128
N_CORES = 8
TOK = B * S                      # 131072 tokens
TOK_PER_CORE = TOK // N_CORES    # 16384
P = 128                          # SBUF partitions

# offsets of the one-hot output fields
OUT_LO, OUT_HI = 68, 84


def build_program(tok_per_core=TOK_PER_CORE, t_per_chunk=32):
    """Build + compile the single-core SPMD Bass program.

    Layout: the core's [tok_per_core, 128] slab is processed in chunks of
    128*T tokens.  Each chunk is one contiguous DRAM block DMA'd to an
    SBUF tile [128, T*128] (partition p holds T consecutive tokens).
    """
    import concourse.bass as bass  # noqa: F401
    from concourse import bacc, mybir, tile

    f32 = mybir.dt.float32
    bf16 = mybir.dt.bfloat16
    i32 = mybir.dt.int32
    Op = mybir.AluOpType
    X = mybir.AxisListType.X

    T = t_per_chunk
    chunk_tok = P * T
    assert tok_per_core % chunk_tok == 0
    n_chunks = tok_per_core // chunk_tok

    nc = bacc.Bacc(
        "TRN2",
        target_bir_lowering=False,
        debug=False,
        enable_asserts=True,
        num_devices=N_CORES,
    )
    x_dram = nc.dram_tensor("x", [tok_per_core, D], f32, kind="ExternalInput").ap()
    y_dram = nc.dram_tensor("y", [tok_per_core, D], f32, kind="ExternalOutput").ap()

    with tile.TileContext(nc) as tc:
        with (
            tc.tile_pool(name="consts", bufs=1) as cpool,
            tc.tile_pool(name="xtiles", bufs=3) as xpool,
            tc.tile_pool(name="tmp", bufs=2) as tp,
        ):
            v = nc.vector

            # --- constants: iota 0..15 (bf16) and iota-16 ------------------
            idx_i = cpool.tile([P, 16], i32)
            nc.gpsimd.iota(idx_i[:], [[1, 16]], base=0, channel_multiplier=0)
            idx16 = cpool.tile([P, 16], bf16)
            v.tensor_copy(idx16[:], idx_i[:])
            idxm16 = cpool.tile([P, 16], bf16)
            v.tensor_scalar(idxm16[:], idx16[:], -16.0, None, Op.add)

            idxm16_b = idxm16.unsqueeze(1).unsqueeze(1).broadcast_to([P, T, 4, 16])
            idx16_b = idx16.unsqueeze(1).broadcast_to([P, T, 16])

            for i in range(n_chunks):
                xt = xpool.tile([P, T * D], f32, name="xt")
                src = x_dram[i * chunk_tok : (i + 1) * chunk_tok, :].rearrange(
                    "(p t) f -> p (t f)", p=P
                )
                nc.sync.dma_start(xt[:], src)

                x3 = xt.rearrange("p (t f) -> p t f", f=D)
                # the four 16-wide argmax fields: g=0 a_lo, 1 a_hi, 2 b_lo, 3 b_hi
                nib = x3[:, :, 4:68].rearrange("p t (g n) -> p t g n", n=16)

                # --- first-occurrence argmax of each field -----------------
                rmax = tp.tile([P, T * 4], f32, name="rmax")
                rmax3 = rmax.rearrange("p (t g) -> p t g", g=4)
                v.tensor_reduce(rmax3, nib, axis=X, op=Op.max)

                eqm = tp.tile([P, T * 64], bf16, name="eqm")
                eqm4 = eqm.rearrange("p (t g n) -> p t g n", g=4, n=16)
                v.tensor_tensor(
                    eqm4, nib, rmax3.unsqueeze(3).broadcast_to([P, T, 4, 16]), Op.is_ge
                )

                cand = tp.tile([P, T * 64], bf16, name="cand")
                cand4 = cand.rearrange("p (t g n) -> p t g n", g=4, n=16)
                v.tensor_tensor(cand4, eqm4, idxm16_b, Op.mult)

                # am = argmax - 16 (reduce_min picks the FIRST maximal index)
                am = tp.tile([P, T * 4], bf16, name="am")
                am3 = am.rearrange("p (t g) -> p t g", g=4)
                v.tensor_reduce(am3, cand4, axis=X, op=Op.min)

                # --- bit-extract the four 4-bit indices --------------------
                def tmp3(nm):
                    t_ = tp.tile([P, T * 4], bf16, name=nm)
                    return t_.rearrange("p (t g) -> p t g", g=4)

                nv = tmp3("nv")     # nibble value 0..15
                v.tensor_scalar(nv, am3, 16.0, None, Op.add)
                b3 = tmp3("b3")
                v.tensor_scalar(b3, nv, 8.0, None, Op.is_ge)
                v2 = tmp3("v2")
                v.scalar_tensor_tensor(v2, b3, -8.0, nv, Op.mult, Op.add)
                b2 = tmp3("b2")
                v.tensor_scalar(b2, v2, 4.0, None, Op.is_ge)
                v1 = tmp3("v1")
                v.scalar_tensor_tensor(v1, b2, -4.0, v2, Op.mult, Op.add)
                b1 = tmp3("b1")
                v.tensor_scalar(b1, v1, 2.0, None, Op.is_ge)
                b0 = tmp3("b0")
                v.scalar_tensor_tensor(b0, b1, -2.0, v1, Op.mult, Op.add)

                # --- flags -> alpha, beta, inactive-offset g ---------------
                def tmp1(nm):
                    t_ = tp.tile([P, T], bf16, name=nm)
                    return t_.unsqueeze(2)  # [P, T, 1]

                ia = tmp1("ia")
                v.tensor_scalar(ia, x3[:, :, 1:2], 0.5, None, Op.is_gt)
                io = tmp1("io")
                v.tensor_scalar(io, x3[:, :, 2:3], 0.5, None, Op.is_gt)
                ix = tmp1("ix")
                v.tensor_scalar(ix, x3[:, :, 3:4], 0.5, None, Op.is_gt)
                mk = tmp1("mk")
                v.tensor_scalar(mk, x3[:, :, 0:1], 0.5, None, Op.is_ge)

                alpha = tmp1("alpha")     # 1 - is_and
                v.tensor_scalar(alpha, ia, -1.0, 1.0, Op.mult, Op.add)
                # beta = is_and*(3-is_or) + (is_or-2)  -> 1 / -1 / -2
                s1 = tmp1("s1")
                v.tensor_scalar(s1, io, -1.0, 3.0, Op.mult, Op.add)
                s2 = tmp1("s2")
                v.tensor_tensor(s2, ia, s1, Op.mult)
                s3 = tmp1("s3")
                v.tensor_scalar(s3, io, -2.0, None, Op.add)
                beta = tmp1("beta")
                v.tensor_tensor(beta, s2, s3, Op.add)
                # g = 16*(1 - mark*anyflag): pushes res out of 0..15 if inactive
                anyf = tmp1("anyf")
                v.tensor_tensor(anyf, ia, io, Op.max)
                anyf2 = tmp1("anyf2")
                v.tensor_tensor(anyf2, anyf, ix, Op.max)
                act = tmp1("act")
                v.tensor_tensor(act, mk, anyf2, Op.mult)
                gof = tmp1("gof")
                v.tensor_scalar(gof, act, -16.0, 16.0, Op.mult, Op.add)

                # --- res = alpha*(a+b) + beta*AND(a,b) + g, lo & hi --------
                def tmp2(nm):
                    t_ = tp.tile([P, T * 2], bf16, name=nm)
                    return t_.rearrange("p (t f) -> p t f", f=2)

                # AND via bits: q = sum 2^k a_k b_k   ([:, :, 0:2]=a, 2:4=b)
                q0 = tmp2("q0")
                v.tensor_tensor(q0, b0[:, :, 0:2], b0[:, :, 2:4], Op.mult)
                q1 = tmp2("q1")
                v.tensor_tensor(q1, b1[:, :, 0:2], b1[:, :, 2:4], Op.mult)
                q2 = tmp2("q2")
                v.tensor_tensor(q2, b2[:, :, 0:2], b2[:, :, 2:4], Op.mult)
                q3 = tmp2("q3")
                v.tensor_tensor(q3, b3[:, :, 0:2], b3[:, :, 2:4], Op.mult)
                qa = tmp2("qa")
                v.scalar_tensor_tensor(qa, q1, 2.0, q0, Op.mult, Op.add)
                qb = tmp2("qb")
                v.scalar_tensor_tensor(qb, q2, 4.0, qa, Op.mult, Op.add)
                qq = tmp2("qq")
                v.scalar_tensor_tensor(qq, q3, 8.0, qb, Op.mult, Op.add)

                ss = tmp2("ss")
                v.tensor_tensor(ss, nv[:, :, 0:2], nv[:, :, 2:4], Op.add)

                t1 = tmp2("t1")
                v.tensor_tensor(t1, ss, alpha.broadcast_to([P, T, 2]), Op.mult)
                t2 = tmp2("t2")
                v.tensor_tensor(t2, qq, beta.broadcast_to([P, T, 2]), Op.mult)
                res = tmp2("res")
                v.tensor_tensor(res, t1, t2, Op.add)
                resg = tmp2("resg")
                v.tensor_tensor(resg, res, gof.broadcast_to([P, T, 2]), Op.add)

                # --- one-hot += 2.0 into the output fields -----------------
                for h, off in enumerate((OUT_LO, OUT_HI)):
                    eqh = tp.tile([P, T * 16], bf16, name=f"eqh{h}")
                    eqh3 = eqh.rearrange("p (t n) -> p t n", n=16)
                    v.tensor_tensor(
                        eqh3,
                        idx16_b,
                        resg[:, :, h : h + 1].broadcast_to([P, T, 16]),
                        Op.is_equal,
                    )
                    xs = x3[:, :, off : off + 16]
                    v.scalar_tensor_tensor(xs, eqh3, 2.0, xs, Op.mult, Op.add)

                dst = y_dram[i * chunk_tok : (i + 1) * chunk_tok, :].rearrange(
                    "(p t) f -> p (t f)", p=P
                )
                nc.sync.dma_start(dst, xt[:])

    nc.compile()
    return nc


_compiled = None


def _get_compiled():
    global _compiled
    if _compiled is None:
        _compiled = build_program()
    return _compiled


def run_on_hw(nc, shards, trace=False, **kw):
    from concourse.bass_utils import run_bass_kernel_spmd

    return run_bass_kernel_spmd(
        nc, [{"x": s} for s in shards], list(range(N_CORES)), trace=trace, **kw
    )


def kernel(x_bd, and_table=None, or_table=None, xor_table=None):
    x = np.ascontiguousarray(np.asarray(x_bd, dtype=np.float32)).reshape(TOK, D)
    shards = [
        np.ascontiguousarray(x[c * TOK_PER_CORE : (c + 1) * TOK_PER_CORE])
        for c in range(N_CORES)
    ]
    nc = _get_compiled()
    res = run_on_hw(nc, shards)
    out = np.concatenate([res.results[c]["y"] for c in range(N_CORES)], axis=0)
    return out.reshape(B, S, D).astype(np.float32)
